# revision 7
# baseline (speedup 1.0000x reference)
"""Multi-Head Latent Attention forward on 8 trn2 NeuronCores (Bass/Tile).

Sharding: core c handles (batch b = c//2, head-half hh = c%2).  Each core
computes, for its batch's full sequence, the shared latent downsample, the
K/V upsample + Q projection for its 8 heads, full softmax attention, and a
partial (8-head) output projection.  Host sums the two head-half partials
per batch.  b_o is added on device by the hh==0 cores (the hh==1 cores
receive a zero bias).

On-chip layout is "transposed activation" land: activations are stored
[feature, seq] so every matmul contraction lands on the partition dim and
no transposes are needed anywhere:

  xT[dm,s] -> latentsT[l,s] -> keysT[d,s] (per head)          (PE)
           -> qT[hd,s] (spilled to DRAM, reloaded per head)   (PE)
  scoresT[k,q] = keysT.T @ qT        exp on ACT (scale=1/sqrt(dh))
  ctxT[d,q]   = values.T @ exp       values built [s,d] natural
  sums[1,q]   = ones.T @ exp         (PSUM-accumulated, M=1 matmuls)
  out[q,dm]   = ctxT.T @ W_o + b_o   (natural layout, contiguous DMA out)

All matmul operands are float32r (full PE rate at N>=256, ~tf32 rounding);
PSUM accumulation is fp32.  Phases:
  P0a latents (stream xT), P0b q (stream xT again, spill qT to DRAM),
  P1 per 4-head group: K/V build then attention (ctxT spilled to DRAM),
  P2 output projection streaming ctxT back per q-stripe.
"""

import numpy as np

B, S, DM, DL, H, DH = 4, 2048, 2048, 512, 16, 128
HH = 8           # heads per core
N_CORES = 8
QT = 256         # query tile (attention)
ST = 256         # seq stripe (projections)
INV_SQRT_DH = 1.0 / np.sqrt(np.float32(DH))

_cache = {}


def _build():
    import concourse.bass as bass
    import concourse.mybir as mybir
    import concourse.tile as tile
    from concourse import bacc

    dt = mybir.dt
    f32, f32r = dt.float32, dt.float32r
    AF = mybir.ActivationFunctionType

    nc = bacc.Bacc("TRN2", target_bir_lowering=False, debug=False,
                   num_devices=N_CORES)

    xT = nc.dram_tensor("xT", [DM, S], f32r, kind="ExternalInput")
    w_down = nc.dram_tensor("w_down", [DM, DL], f32r, kind="ExternalInput")
    w_q = nc.dram_tensor("w_q", [DM, HH * DH], f32r, kind="ExternalInput")
    w_uk = nc.dram_tensor("w_uk", [DL, HH * DH], f32r, kind="ExternalInput")
    w_uv = nc.dram_tensor("w_uv", [DL, HH * DH], f32r, kind="ExternalInput")
    w_o = nc.dram_tensor("w_o", [HH * DH, DM], f32r, kind="ExternalInput")
    b_down = nc.dram_tensor("b_down", [DL], f32, kind="ExternalInput")
    b_q = nc.dram_tensor("b_q", [HH * DH], f32, kind="ExternalInput")
    b_uk = nc.dram_tensor("b_uk", [HH * DH], f32, kind="ExternalInput")
    b_uv = nc.dram_tensor("b_uv", [HH * DH], f32, kind="ExternalInput")
    b_o = nc.dram_tensor("b_o", [DM], f32r, kind="ExternalInput")
    out = nc.dram_tensor("out", [S, DM], f32, kind="ExternalOutput")

    qT_s = nc.dram_tensor("qT_s", [HH * DH, S], f32r)      # scratch
    ctxT_s = nc.dram_tensor("ctxT_s", [HH * DH, S], f32r)  # scratch

    n_st = S // ST          # 8 stripes
    n_qt = S // QT          # 8 q tiles per head
    n_kc = S // 128         # 16 key chunks
    n_dmc = DM // 128       # 16 model-dim chunks
    n_lc = DL // 128        # 4 latent chunks

    xT_r = xT.rearrange("(t p) s -> p t s", p=128)
    qTs_r = qT_s.rearrange("(t p) s -> p t s", p=128)
    ctxs_r = ctxT_s.rearrange("(t p) s -> p t s", p=128)
    wo_r = w_o.rearrange("(t p) d -> p t d", p=128)

    with tile.TileContext(nc, pool_alloc_mode="queue") as tc:
        with tc.tile_pool(name="persist", bufs=1) as pp:
            latentsT = pp.tile([128, n_lc, S], f32r, tag="latT")
            onescol = pp.tile([128, 1], f32r, tag="ones_c")
            onesrow = pp.tile([1, 512], f32r, tag="ones_r")
            ones_f = pp.tile([128, 512], f32, tag="ones_f")
            buv_sb = pp.tile([128, HH], f32, tag="buv")
            nc.gpsimd.memset(ones_f[:], 1.0)
            nc.vector.tensor_copy(onescol[:], ones_f[:, 0:1])
            nc.vector.tensor_copy(onesrow[:], ones_f[0:1, :])
            nc.sync.dma_start(buv_sb[:], b_uv.rearrange("(h p) -> p h", p=128))

            # ---------------- P0a: latentsT (one pass over xT) ------------
            # w_q/w_uk prefetch overlaps the latents compute.
            with tc.tile_pool(name="pkv", bufs=1) as pkvp:
              with tc.tile_pool(name="pq", bufs=1) as pqp:
                with tc.tile_pool(name="p0w", bufs=1) as wp, \
                     tc.tile_pool(name="p0x", bufs=2) as xp, \
                     tc.tile_pool(name="p0ps", bufs=3, space="PSUM") as pps:
                    wdown_sb = wp.tile([128, n_dmc, DL], f32r, tag="wdown")
                    bdown_sb = wp.tile([128, n_lc], f32, tag="bdown")
                    nc.sync.dma_start(wdown_sb[:], w_down.rearrange("(t p) l -> p t l", p=128))
                    nc.sync.dma_start(bdown_sb[:], b_down.rearrange("(t p) -> p t", p=128))
                    wq_sb = pqp.tile([128, n_dmc, HH * DH], f32r, tag="wq")
                    bq_sb = pqp.tile([128, HH], f32, tag="bq")
                    nc.sync.dma_start(wq_sb[:], w_q.rearrange("(t p) d -> p t d", p=128))
                    nc.sync.dma_start(bq_sb[:], b_q.rearrange("(t p) -> p t", p=128))

                    for st in range(n_st):
                        ssl = bass.ts(st, ST)
                        xt = xp.tile([128, n_dmc, ST], f32r, tag="xt")
                        nc.sync.dma_start(xt[:], xT_r[:, :, ssl])
                        for lt in range(n_lc):
                            ps = pps.tile([128, ST], f32, tag="lat")
                            for c in range(n_dmc):
                                nc.tensor.matmul(ps[:], wdown_sb[:, c, bass.ts(lt, 128)],
                                                 xt[:, c, :], start=(c == 0), stop=(c == n_dmc - 1))
                            nc.scalar.activation(latentsT[:, lt, ssl], ps[:], AF.Identity,
                                                 bias=bdown_sb[:, lt:lt + 1])

                # ------------ P0b: q projection (second pass over xT) -----
                with tc.tile_pool(name="p0bx", bufs=2) as xp2, \
                     tc.tile_pool(name="p0bs", bufs=2) as sp, \
                     tc.tile_pool(name="p0bps", bufs=3, space="PSUM") as pps2:
                    wuk_sb = pkvp.tile([128, n_lc, HH * DH], f32r, tag="wuk")
                    buk_sb = pkvp.tile([128, HH], f32, tag="buk")
                    nc.sync.dma_start(wuk_sb[:], w_uk.rearrange("(t p) d -> p t d", p=128))
                    nc.sync.dma_start(buk_sb[:], b_uk.rearrange("(t p) -> p t", p=128))

                    for st in range(n_st):
                        ssl = bass.ts(st, ST)
                        xt = xp2.tile([128, n_dmc, ST], f32r, tag="xt2")
                        nc.sync.dma_start(xt[:], xT_r[:, :, ssl])
                        qstage = sp.tile([128, HH, ST], f32r, tag="qs")
                        for ht in range(HH):
                            ps = pps2.tile([128, ST], f32, tag="q")
                            for c in range(n_dmc):
                                nc.tensor.matmul(ps[:], wq_sb[:, c, bass.ts(ht, 128)],
                                                 xt[:, c, :], start=(c == 0), stop=(c == n_dmc - 1))
                            nc.scalar.activation(qstage[:, ht, :], ps[:], AF.Identity,
                                                 bias=bq_sb[:, ht:ht + 1])
                        nc.sync.dma_start(qTs_r[:, :, ssl], qstage[:])

              # ------------ P1: per 4-head group: K/V build + attention --
              with tc.tile_pool(name="p1kv", bufs=1) as kvp, \
                   tc.tile_pool(name="p1q", bufs=2) as qp, \
                   tc.tile_pool(name="p1e", bufs=2) as ep, \
                   tc.tile_pool(name="p1s", bufs=3) as sp2:
                    keysT = kvp.tile([128, 4, S], f32r, tag="keysT")
                    values = kvp.tile([128, n_kc, 512], f32r, tag="values")
                    wuv_sb = kvp.tile([128, n_lc, HH * DH], f32r, tag="wuv")
                    nc.sync.dma_start(wuv_sb[:], w_uv.rearrange("(t p) d -> p t d", p=128))

                    for g in range(2):
                        gsl = bass.ds(g * 512, 512)
                        with tc.tile_pool(name=f"kvps{g}", bufs=2, space="PSUM") as kvps:
                            for dt_ in range(4):
                                for st in range(n_st):
                                    ps = kvps.tile([128, ST], f32, tag="kv")
                                    for lt in range(n_lc):
                                        nc.tensor.matmul(
                                            ps[:], wuk_sb[:, lt, bass.ds(g * 512 + dt_ * 128, 128)],
                                            latentsT[:, lt, bass.ts(st, ST)],
                                            start=(lt == 0), stop=(lt == n_lc - 1))
                                    nc.scalar.activation(keysT[:, dt_, bass.ts(st, ST)], ps[:],
                                                         AF.Identity,
                                                         bias=buk_sb[:, g * 4 + dt_:g * 4 + dt_ + 1])
                            for sc in range(n_kc):
                                ps = kvps.tile([128, 512], f32, tag="kv")
                                for lt in range(n_lc):
                                    nc.tensor.matmul(ps[:], latentsT[:, lt, bass.ts(sc, 128)],
                                                     wuv_sb[:, lt, gsl],
                                                     start=(lt == 0), stop=(lt == n_lc - 1))
                                nc.scalar.activation(values[:, sc, :], ps[:], AF.Identity)

                        with tc.tile_pool(name=f"scp{g}", bufs=2, space="PSUM") as scp, \
                             tc.tile_pool(name=f"avp{g}", bufs=2, space="PSUM") as avps, \
                             tc.tile_pool(name=f"smp{g}", bufs=1, space="PSUM") as smps, \
                             tc.tile_pool(name=f"bcp{g}", bufs=1, space="PSUM") as bcps:
                            for hl in range(4):
                                h = g * 4 + hl
                                qh = qp.tile([128, S], f32r, tag="qh")
                                nc.sync.dma_start(qh[:], qT_s[bass.ts(h, 128), :])
                                for qt in range(n_qt):
                                    qsl = bass.ts(qt, QT)
                                    eb = ep.tile([128, n_kc, QT], f32r, tag="eb")
                                    for blk in range(4):
                                        ps = scp.tile([128, 4, QT], f32, tag="sc")
                                        for c4 in range(4):
                                            c = blk * 4 + c4
                                            nc.tensor.matmul(ps[:, c4, :],
                                                             keysT[:, hl, bass.ts(c, 128)],
                                                             qh[:, qsl], start=True, stop=True)
                                        nc.scalar.activation(eb[:, blk * 4:(blk + 1) * 4, :],
                                                             ps[:], AF.Exp, scale=INV_SQRT_DH)
                                    avp = avps.tile([128, QT], f32, tag="av")
                                    for c in range(n_kc):
                                        nc.tensor.matmul(avp[:], values[:, c, bass.ts(hl, 128)],
                                                         eb[:, c, :], start=(c == 0),
                                                         stop=(c == n_kc - 1))
                                    smp = smps.tile([1, QT], f32, tag="sm")
                                    for c in range(n_kc):
                                        nc.tensor.matmul(smp[:], onescol[:], eb[:, c, :],
                                                         start=(c == 0), stop=(c == n_kc - 1))
                                    recip = sp2.tile([1, QT], f32, tag="rc")
                                    nc.vector.reciprocal_approx_fast(recip[:], smp[:])
                                    recipr = sp2.tile([1, QT], f32r, tag="rcr")
                                    nc.vector.tensor_scalar_add(recipr[:], recip[:], 0.0)
                                    bcp = bcps.tile([128, QT], f32, tag="bc")
                                    nc.tensor.matmul(bcp[:], onesrow[:, :128], recipr[:],
                                                     start=True, stop=True)
                                    bcs = sp2.tile([128, QT], f32, tag="bcs")
                                    nc.vector.tensor_copy(bcs[:], bcp[:])
                                    tmp = sp2.tile([128, QT], f32, tag="tmp")
                                    nc.vector.tensor_mul(tmp[:], avp[:], bcs[:])
                                    ctxs = sp2.tile([128, QT], f32r, tag="ctxs")
                                    nc.vector.tensor_scalar_add(ctxs[:], tmp[:],
                                                                buv_sb[:, h:h + 1])
                                    nc.sync.dma_start(ctxT_s[bass.ts(h, 128), qsl], ctxs[:])

            # ---------------- P2: output projection (natural layout) ------
            # W_o resident in 4 chunks (prefetched in order); ctxT streamed
            # back per 128-query stripe.
            with tc.tile_pool(name="p2w", bufs=1) as wop, \
                 tc.tile_pool(name="p2c", bufs=3) as cp, \
                 tc.tile_pool(name="p2s", bufs=3) as osp, \
                 tc.tile_pool(name="p2ps", bufs=3, space="PSUM") as ops:
                wo_t = []
                for dmt in range(4):
                    w = wop.tile([128, HH, 512], f32r, tag=f"wo{dmt}")
                    nc.sync.dma_start(w[:], wo_r[:, :, bass.ts(dmt, 512)])
                    wo_t.append(w)
                bo_sb = wop.tile([1, DM], f32r, tag="bo")
                nc.sync.dma_start(bo_sb[:], b_o[None, :])
                for qt in range(S // 128):
                    qsl = bass.ts(qt, 128)
                    ctx_t = cp.tile([128, HH, 128], f32r, tag="ctxq")
                    nc.sync.dma_start(ctx_t[:], ctxs_r[:, :, qsl])
                    for dmt in range(4):
                        dsl = bass.ts(dmt, 512)
                        ps = ops.tile([128, 512], f32, tag="o")
                        for hc in range(HH):
                            nc.tensor.matmul(ps[:], ctx_t[:, hc, :],
                                             wo_t[dmt][:, hc, :], start=(hc == 0), stop=False)
                        nc.tensor.matmul(ps[:], onesrow[:, :128], bo_sb[:, dsl],
                                         start=False, stop=True)
                        ostage = osp.tile([128, 512], f32, tag="os")
                        nc.vector.tensor_copy(ostage[:], ps[:])
                        nc.sync.dma_start(out[qsl, dsl], ostage[:])

    nc.compile()
    return nc


def _get_nc():
    if "nc" not in _cache:
        _cache["nc"] = _build()
    return _cache["nc"]


def _in_maps(x, W_down, b_down, W_uk, b_uk, W_uv, b_uv, W_q, b_q, W_o, b_o):
    x = np.asarray(x, dtype=np.float32)
    zeros_bo = np.zeros_like(np.asarray(b_o))
    maps = []
    for c in range(N_CORES):
        b, hh = c // 2, c % 2
        hsl = slice(hh * HH * DH, (hh + 1) * HH * DH)
        maps.append({
            "xT": np.ascontiguousarray(x[b].T),
            "w_down": np.asarray(W_down),
            "w_q": np.ascontiguousarray(np.asarray(W_q)[:, hsl]),
            "w_uk": np.ascontiguousarray(np.asarray(W_uk)[:, hsl]),
            "w_uv": np.ascontiguousarray(np.asarray(W_uv)[:, hsl]),
            "w_o": np.ascontiguousarray(np.asarray(W_o)[hsl, :]),
            "b_down": np.asarray(b_down),
            "b_q": np.ascontiguousarray(np.asarray(b_q)[hsl]),
            "b_uk": np.ascontiguousarray(np.asarray(b_uk)[hsl]),
            "b_uv": np.ascontiguousarray(np.asarray(b_uv)[hsl]),
            "b_o": np.asarray(b_o) if hh == 0 else zeros_bo,
        })
    return maps


def kernel(x, W_down, b_down, W_uk, b_uk, W_uv, b_uv, W_q, b_q, W_o, b_o):
    from concourse.bass_utils import run_bass_kernel_spmd

    nc = _get_nc()
    maps = _in_maps(x, W_down, b_down, W_uk, b_uk, W_uv, b_uv, W_q, b_q, W_o, b_o)
    res = run_bass_kernel_spmd(nc, maps, list(range(N_CORES)))
    full = np.empty((B, S, DM), np.float32)
    for b in range(B):
        full[b] = res.results[2 * b]["out"] + res.results[2 * b + 1]["out"]
    return full


# revision 9
# speedup vs baseline: 1.0446x; 1.0446x over previous
"""Multi-Head Latent Attention forward on 8 trn2 NeuronCores (Bass/Tile).

Sharding: core c handles (batch b = c//2, head-half hh = c%2).  Each core
computes, for its batch's full sequence, the shared latent downsample, the
K/V upsample + Q projection for its 8 heads, full softmax attention, and a
partial (8-head) output projection.  Host sums the two head-half partials
per batch.  b_o is added on device by the hh==0 cores (the hh==1 cores
receive a zero bias).

On-chip layout is "transposed activation" land: activations are stored
[feature, seq] so every matmul contraction lands on the partition dim and
no transposes are needed anywhere:

  xT[dm,s] -> latentsT[l,s] -> keysT[d,s] (per head)          (PE)
           -> qT[hd,s] (spilled to DRAM, reloaded per head)   (PE)
  scoresT[k,q] = keysT.T @ qT        exp on ACT (scale=1/sqrt(dh))
  ctxT[d,q]   = values.T @ exp       values built [s,d] natural
  sums[1,q]   = ones.T @ exp         (PSUM-accumulated, M=1 matmuls)
  out[q,dm]   = ctxT.T @ W_o + b_o   (natural layout, contiguous DMA out)

All matmul operands are float32r (full PE rate at N>=256, ~tf32 rounding);
PSUM accumulation is fp32.  Phases:
  P0a latents (stream xT), P0b q (stream xT again, spill qT to DRAM),
  P1 per 4-head group: K/V build then attention (ctxT spilled to DRAM),
  P2 output projection streaming ctxT back per q-stripe.
"""

import numpy as np

B, S, DM, DL, H, DH = 4, 2048, 2048, 512, 16, 128
HH = 8           # heads per core
N_CORES = 8
QT = 256         # query tile (attention)
ST = 256         # seq stripe (projections)
INV_SQRT_DH = 1.0 / np.sqrt(np.float32(DH))

_cache = {}


def _build():
    import concourse.bass as bass
    import concourse.mybir as mybir
    import concourse.tile as tile
    from concourse import bacc

    dt = mybir.dt
    f32, f32r = dt.float32, dt.float32r
    AF = mybir.ActivationFunctionType

    nc = bacc.Bacc("TRN2", target_bir_lowering=False, debug=False,
                   num_devices=N_CORES)

    xT = nc.dram_tensor("xT", [DM, S], f32r, kind="ExternalInput")
    w_down = nc.dram_tensor("w_down", [DM, DL], f32r, kind="ExternalInput")
    w_q = nc.dram_tensor("w_q", [DM, HH * DH], f32r, kind="ExternalInput")
    w_uk = nc.dram_tensor("w_uk", [DL, HH * DH], f32r, kind="ExternalInput")
    w_uv = nc.dram_tensor("w_uv", [DL, HH * DH], f32r, kind="ExternalInput")
    w_o = nc.dram_tensor("w_o", [HH * DH, DM], f32r, kind="ExternalInput")
    b_down = nc.dram_tensor("b_down", [DL], f32, kind="ExternalInput")
    b_q = nc.dram_tensor("b_q", [HH * DH], f32, kind="ExternalInput")
    b_uk = nc.dram_tensor("b_uk", [HH * DH], f32, kind="ExternalInput")
    b_uv = nc.dram_tensor("b_uv", [HH * DH], f32, kind="ExternalInput")
    b_o = nc.dram_tensor("b_o", [DM], f32r, kind="ExternalInput")
    out = nc.dram_tensor("out", [S, DM], f32, kind="ExternalOutput")

    qT_s = nc.dram_tensor("qT_s", [HH * DH, S], f32r)      # scratch
    ctxT_s = nc.dram_tensor("ctxT_s", [HH * DH, S], f32r)  # scratch

    n_st = S // ST          # 8 stripes
    n_qt = S // QT          # 8 q tiles per head
    n_kc = S // 128         # 16 key chunks
    n_dmc = DM // 128       # 16 model-dim chunks
    n_lc = DL // 128        # 4 latent chunks

    xT_r = xT.rearrange("(t p) s -> p t s", p=128)
    qTs_r = qT_s.rearrange("(t p) s -> p t s", p=128)
    ctxs_r = ctxT_s.rearrange("(t p) s -> p t s", p=128)
    wo_r = w_o.rearrange("(t p) d -> p t d", p=128)

    with tile.TileContext(nc, pool_alloc_mode="queue") as tc:
        with tc.tile_pool(name="persist", bufs=1) as pp:
            latentsT = pp.tile([128, n_lc, S], f32r, tag="latT")
            onescol = pp.tile([128, 1], f32r, tag="ones_c")
            onesrow = pp.tile([1, 512], f32r, tag="ones_r")
            ones_f = pp.tile([128, 512], f32, tag="ones_f")
            buv_sb = pp.tile([128, HH], f32, tag="buv")
            nc.gpsimd.memset(ones_f[:], 1.0)
            nc.vector.tensor_copy(onescol[:], ones_f[:, 0:1])
            nc.vector.tensor_copy(onesrow[:], ones_f[0:1, :])
            nc.gpsimd.dma_start(buv_sb[:], b_uv.rearrange("(h p) -> p h", p=128))

            # ---------------- P0a: latentsT (one pass over xT) ------------
            # w_q/w_uk prefetch overlaps the latents compute.
            with tc.tile_pool(name="pkv", bufs=1) as pkvp:
              with tc.tile_pool(name="pq", bufs=1) as pqp:
                with tc.tile_pool(name="p0w", bufs=1) as wp, \
                     tc.tile_pool(name="p0x", bufs=2) as xp, \
                     tc.tile_pool(name="p0ps", bufs=3, space="PSUM") as pps:
                    wdown_sb = wp.tile([128, n_dmc, DL], f32r, tag="wdown")
                    bdown_sb = wp.tile([128, n_lc], f32, tag="bdown")
                    nc.gpsimd.dma_start(wdown_sb[:], w_down.rearrange("(t p) l -> p t l", p=128))
                    nc.gpsimd.dma_start(bdown_sb[:], b_down.rearrange("(t p) -> p t", p=128))
                    wq_sb = pqp.tile([128, n_dmc, HH * DH], f32r, tag="wq")
                    bq_sb = pqp.tile([128, HH], f32, tag="bq")
                    nc.gpsimd.dma_start(wq_sb[:], w_q.rearrange("(t p) d -> p t d", p=128))
                    nc.gpsimd.dma_start(bq_sb[:], b_q.rearrange("(t p) -> p t", p=128))

                    for st in range(n_st):
                        ssl = bass.ts(st, ST)
                        xt = xp.tile([128, n_dmc, ST], f32r, tag="xt")
                        nc.sync.dma_start(xt[:], xT_r[:, :, ssl])
                        for lt in range(n_lc):
                            ps = pps.tile([128, ST], f32, tag="lat")
                            for c in range(n_dmc):
                                nc.tensor.matmul(ps[:], wdown_sb[:, c, bass.ts(lt, 128)],
                                                 xt[:, c, :], start=(c == 0), stop=(c == n_dmc - 1))
                            nc.scalar.activation(latentsT[:, lt, ssl], ps[:], AF.Identity,
                                                 bias=bdown_sb[:, lt:lt + 1])

                # ------------ P0b: q projection (second pass over xT) -----
                with tc.tile_pool(name="p0bx", bufs=2) as xp2, \
                     tc.tile_pool(name="p0bs", bufs=2) as sp, \
                     tc.tile_pool(name="p0bps", bufs=3, space="PSUM") as pps2:
                    wuk_sb = pkvp.tile([128, n_lc, HH * DH], f32r, tag="wuk")
                    buk_sb = pkvp.tile([128, HH], f32, tag="buk")
                    nc.gpsimd.dma_start(wuk_sb[:], w_uk.rearrange("(t p) d -> p t d", p=128))
                    nc.gpsimd.dma_start(buk_sb[:], b_uk.rearrange("(t p) -> p t", p=128))

                    for st in range(n_st):
                        ssl = bass.ts(st, ST)
                        xt = xp2.tile([128, n_dmc, ST], f32r, tag="xt2")
                        nc.sync.dma_start(xt[:], xT_r[:, :, ssl])
                        qstage = sp.tile([128, HH, ST], f32r, tag="qs")
                        for ht in range(HH):
                            ps = pps2.tile([128, ST], f32, tag="q")
                            for c in range(n_dmc):
                                nc.tensor.matmul(ps[:], wq_sb[:, c, bass.ts(ht, 128)],
                                                 xt[:, c, :], start=(c == 0), stop=(c == n_dmc - 1))
                            nc.scalar.activation(qstage[:, ht, :], ps[:], AF.Identity,
                                                 bias=bq_sb[:, ht:ht + 1])
                        nc.sync.dma_start(qTs_r[:, :, ssl], qstage[:])

              # ------------ P1: per 4-head group: K/V build + attention --
              with tc.tile_pool(name="p1kv", bufs=1) as kvp, \
                   tc.tile_pool(name="p1q", bufs=2) as qp, \
                   tc.tile_pool(name="p1e", bufs=2) as ep, \
                   tc.tile_pool(name="p1s", bufs=3) as sp2:
                    keysT = kvp.tile([128, 4, S], f32r, tag="keysT")
                    values = kvp.tile([128, n_kc, 512], f32r, tag="values")
                    wuv_sb = kvp.tile([128, n_lc, HH * DH], f32r, tag="wuv")
                    nc.gpsimd.dma_start(wuv_sb[:], w_uv.rearrange("(t p) d -> p t d", p=128))

                    for g in range(2):
                        gsl = bass.ds(g * 512, 512)
                        with tc.tile_pool(name=f"kvps{g}", bufs=2, space="PSUM") as kvps:
                            for dt_ in range(4):
                                for st in range(n_st):
                                    ps = kvps.tile([128, ST], f32, tag="kv")
                                    for lt in range(n_lc):
                                        nc.tensor.matmul(
                                            ps[:], wuk_sb[:, lt, bass.ds(g * 512 + dt_ * 128, 128)],
                                            latentsT[:, lt, bass.ts(st, ST)],
                                            start=(lt == 0), stop=(lt == n_lc - 1))
                                    nc.scalar.activation(keysT[:, dt_, bass.ts(st, ST)], ps[:],
                                                         AF.Identity,
                                                         bias=buk_sb[:, g * 4 + dt_:g * 4 + dt_ + 1])
                            for sc in range(n_kc):
                                ps = kvps.tile([128, 512], f32, tag="kv")
                                for lt in range(n_lc):
                                    nc.tensor.matmul(ps[:], latentsT[:, lt, bass.ts(sc, 128)],
                                                     wuv_sb[:, lt, gsl],
                                                     start=(lt == 0), stop=(lt == n_lc - 1))
                                nc.scalar.activation(values[:, sc, :], ps[:], AF.Identity)

                        with tc.tile_pool(name=f"scp{g}", bufs=2, space="PSUM") as scp, \
                             tc.tile_pool(name=f"avp{g}", bufs=2, space="PSUM") as avps, \
                             tc.tile_pool(name=f"smp{g}", bufs=1, space="PSUM") as smps, \
                             tc.tile_pool(name=f"bcp{g}", bufs=1, space="PSUM") as bcps:
                            for hl in range(4):
                                h = g * 4 + hl
                                qh = qp.tile([128, S], f32r, tag="qh")
                                nc.sync.dma_start(qh[:], qT_s[bass.ts(h, 128), :])
                                for qt in range(n_qt):
                                    qsl = bass.ts(qt, QT)
                                    eb = ep.tile([128, n_kc, QT], f32r, tag="eb")
                                    for blk in range(4):
                                        ps = scp.tile([128, 4, QT], f32, tag="sc")
                                        for c4 in range(4):
                                            c = blk * 4 + c4
                                            nc.tensor.matmul(ps[:, c4, :],
                                                             keysT[:, hl, bass.ts(c, 128)],
                                                             qh[:, qsl], start=True, stop=True)
                                        nc.scalar.activation(eb[:, blk * 4:(blk + 1) * 4, :],
                                                             ps[:], AF.Exp, scale=INV_SQRT_DH)
                                    avp = avps.tile([128, QT], f32, tag="av")
                                    for c in range(n_kc):
                                        nc.tensor.matmul(avp[:], values[:, c, bass.ts(hl, 128)],
                                                         eb[:, c, :], start=(c == 0),
                                                         stop=(c == n_kc - 1))
                                    smp = smps.tile([1, QT], f32, tag="sm")
                                    for c in range(n_kc):
                                        nc.tensor.matmul(smp[:], onescol[:], eb[:, c, :],
                                                         start=(c == 0), stop=(c == n_kc - 1))
                                    recip = sp2.tile([1, QT], f32, tag="rc")
                                    nc.vector.reciprocal_approx_fast(recip[:], smp[:])
                                    recipr = sp2.tile([1, QT], f32r, tag="rcr")
                                    nc.vector.tensor_scalar_add(recipr[:], recip[:], 0.0)
                                    bcp = bcps.tile([128, QT], f32, tag="bc")
                                    nc.tensor.matmul(bcp[:], onesrow[:, :128], recipr[:],
                                                     start=True, stop=True)
                                    bcs = sp2.tile([128, QT], f32, tag="bcs")
                                    nc.vector.tensor_copy(bcs[:], bcp[:])
                                    tmp = sp2.tile([128, QT], f32, tag="tmp")
                                    nc.vector.tensor_mul(tmp[:], avp[:], bcs[:])
                                    ctxs = sp2.tile([128, QT], f32r, tag="ctxs")
                                    nc.vector.tensor_scalar_add(ctxs[:], tmp[:],
                                                                buv_sb[:, h:h + 1])
                                    nc.sync.dma_start(ctxT_s[bass.ts(h, 128), qsl], ctxs[:])

            # ---------------- P2: output projection (natural layout) ------
            # W_o resident in 4 chunks (prefetched in order); ctxT streamed
            # back per 128-query stripe.
            with tc.tile_pool(name="p2w", bufs=1) as wop, \
                 tc.tile_pool(name="p2c", bufs=3) as cp, \
                 tc.tile_pool(name="p2s", bufs=3) as osp, \
                 tc.tile_pool(name="p2ps", bufs=2, space="PSUM") as ops:
                wo_t = []
                for dmt in range(4):
                    w = wop.tile([128, HH, 512], f32r, tag=f"wo{dmt}")
                    nc.sync.dma_start(w[:], wo_r[:, :, bass.ts(dmt, 512)])
                    wo_t.append(w)
                bo_sb = wop.tile([1, DM], f32r, tag="bo")
                nc.sync.dma_start(bo_sb[:], b_o[None, :])
                for qt in range(S // 128):
                    qsl = bass.ts(qt, 128)
                    ctx_t = cp.tile([128, HH, 128], f32r, tag="ctxq")
                    nc.sync.dma_start(ctx_t[:], ctxs_r[:, :, qsl])
                    pst = [ops.tile([128, 512], f32, tag=f"o{dmt}", name=f"ps_o{dmt}") for dmt in range(4)]
                    for hc in range(HH):
                        for dmt in range(4):
                            nc.tensor.matmul(pst[dmt][:], ctx_t[:, hc, :],
                                             wo_t[dmt][:, hc, :], start=(hc == 0), stop=False)
                    for dmt in range(4):
                        dsl = bass.ts(dmt, 512)
                        nc.tensor.matmul(pst[dmt][:], onesrow[:, :128], bo_sb[:, dsl],
                                         start=False, stop=True)
                        ostage = osp.tile([128, 512], f32, tag="os")
                        nc.vector.tensor_copy(ostage[:], pst[dmt][:])
                        nc.sync.dma_start(out[qsl, dsl], ostage[:])

    nc.compile()
    return nc


def _get_nc():
    if "nc" not in _cache:
        _cache["nc"] = _build()
    return _cache["nc"]


def _in_maps(x, W_down, b_down, W_uk, b_uk, W_uv, b_uv, W_q, b_q, W_o, b_o):
    x = np.asarray(x, dtype=np.float32)
    zeros_bo = np.zeros_like(np.asarray(b_o))
    maps = []
    for c in range(N_CORES):
        b, hh = c // 2, c % 2
        hsl = slice(hh * HH * DH, (hh + 1) * HH * DH)
        maps.append({
            "xT": np.ascontiguousarray(x[b].T),
            "w_down": np.asarray(W_down),
            "w_q": np.ascontiguousarray(np.asarray(W_q)[:, hsl]),
            "w_uk": np.ascontiguousarray(np.asarray(W_uk)[:, hsl]),
            "w_uv": np.ascontiguousarray(np.asarray(W_uv)[:, hsl]),
            "w_o": np.ascontiguousarray(np.asarray(W_o)[hsl, :]),
            "b_down": np.asarray(b_down),
            "b_q": np.ascontiguousarray(np.asarray(b_q)[hsl]),
            "b_uk": np.ascontiguousarray(np.asarray(b_uk)[hsl]),
            "b_uv": np.ascontiguousarray(np.asarray(b_uv)[hsl]),
            "b_o": np.asarray(b_o) if hh == 0 else zeros_bo,
        })
    return maps


def kernel(x, W_down, b_down, W_uk, b_uk, W_uv, b_uv, W_q, b_q, W_o, b_o):
    from concourse.bass_utils import run_bass_kernel_spmd

    nc = _get_nc()
    maps = _in_maps(x, W_down, b_down, W_uk, b_uk, W_uv, b_uv, W_q, b_q, W_o, b_o)
    res = run_bass_kernel_spmd(nc, maps, list(range(N_CORES)))
    full = np.empty((B, S, DM), np.float32)
    for b in range(B):
        full[b] = res.results[2 * b]["out"] + res.results[2 * b + 1]["out"]
    return full


# revision 10
# speedup vs baseline: 1.1149x; 1.0673x over previous
"""Multi-Head Latent Attention forward on 8 trn2 NeuronCores (Bass/Tile).

Sharding: core c handles (batch b = c//2, head-half hh = c%2).  Each core
computes, for its batch's full sequence, the shared latent downsample, the
K/V upsample + Q projection for its 8 heads, full softmax attention, and a
partial (8-head) output projection.  Host sums the two head-half partials
per batch.  b_o is added on device by the hh==0 cores (the hh==1 cores
receive a zero bias).

On-chip layout is "transposed activation" land: activations are stored
[feature, seq] so every matmul contraction lands on the partition dim and
no transposes are needed anywhere:

  xT[dm,s] -> latentsT[l,s] -> keysT[d,s] (per head)          (PE)
           -> qT[hd,s] (spilled to DRAM, reloaded per head)   (PE)
  scoresT[k,q] = keysT.T @ qT        exp on ACT (scale=1/sqrt(dh))
  ctxT[d,q]   = values.T @ exp       values built [s,d] natural
  sums[1,q]   = ones.T @ exp         (PSUM-accumulated, M=1 matmuls)
  out[q,dm]   = ctxT.T @ W_o + b_o   (natural layout, contiguous DMA out)

All matmul operands are float32r (full PE rate at N>=256, ~tf32 rounding);
PSUM accumulation is fp32.  Phases:
  P0a latents (stream xT), P0b q (stream xT again, spill qT to DRAM),
  P1 per 4-head group: K/V build then attention (ctxT spilled to DRAM),
  P2 output projection streaming ctxT back per q-stripe.
"""

import numpy as np

B, S, DM, DL, H, DH = 4, 2048, 2048, 512, 16, 128
HH = 8           # heads per core
N_CORES = 8
QT = 256         # query tile (attention)
ST = 256         # seq stripe (projections)
INV_SQRT_DH = 1.0 / np.sqrt(np.float32(DH))

_cache = {}


def _build():
    import concourse.bass as bass
    import concourse.mybir as mybir
    import concourse.tile as tile
    from concourse import bacc

    dt = mybir.dt
    f32, f32r = dt.float32, dt.float32r
    AF = mybir.ActivationFunctionType

    nc = bacc.Bacc("TRN2", target_bir_lowering=False, debug=False,
                   num_devices=N_CORES)

    xT = nc.dram_tensor("xT", [DM, S], f32r, kind="ExternalInput")
    w_down = nc.dram_tensor("w_down", [DM, DL], f32r, kind="ExternalInput")
    w_q = nc.dram_tensor("w_q", [DM, HH * DH], f32r, kind="ExternalInput")
    w_uk = nc.dram_tensor("w_uk", [DL, HH * DH], f32r, kind="ExternalInput")
    w_uv = nc.dram_tensor("w_uv", [DL, HH * DH], f32r, kind="ExternalInput")
    w_o = nc.dram_tensor("w_o", [HH * DH, DM], f32r, kind="ExternalInput")
    b_down = nc.dram_tensor("b_down", [DL], f32, kind="ExternalInput")
    b_q = nc.dram_tensor("b_q", [HH * DH], f32, kind="ExternalInput")
    b_uk = nc.dram_tensor("b_uk", [HH * DH], f32, kind="ExternalInput")
    b_uv = nc.dram_tensor("b_uv", [HH * DH], f32, kind="ExternalInput")
    b_o = nc.dram_tensor("b_o", [DM], f32r, kind="ExternalInput")
    out = nc.dram_tensor("out", [S, DM], f32, kind="ExternalOutput")

    qT_s = nc.dram_tensor("qT_s", [HH * DH, S], f32r)      # scratch
    ctxT_s = nc.dram_tensor("ctxT_s", [HH * DH, S], f32r)  # scratch

    n_st = S // ST          # 8 stripes
    n_qt = S // QT          # 8 q tiles per head
    n_kc = S // 128         # 16 key chunks
    n_dmc = DM // 128       # 16 model-dim chunks
    n_lc = DL // 128        # 4 latent chunks

    xT_r = xT.rearrange("(t p) s -> p t s", p=128)
    qTs_r = qT_s.rearrange("(t p) s -> p t s", p=128)
    ctxs_r = ctxT_s.rearrange("(t p) s -> p t s", p=128)
    wo_r = w_o.rearrange("(t p) d -> p t d", p=128)

    with tile.TileContext(nc, pool_alloc_mode="queue") as tc:
        with tc.tile_pool(name="persist", bufs=1) as pp:
            latentsT = pp.tile([128, n_lc, S], f32r, tag="latT")
            onescol = pp.tile([128, 1], f32r, tag="ones_c")
            onesrow = pp.tile([1, 512], f32r, tag="ones_r")
            ones_f = pp.tile([128, 512], f32, tag="ones_f")
            buv_sb = pp.tile([128, HH], f32, tag="buv")
            nc.gpsimd.memset(ones_f[:], 1.0)
            nc.vector.tensor_copy(onescol[:], ones_f[:, 0:1])
            nc.vector.tensor_copy(onesrow[:], ones_f[0:1, :])
            nc.gpsimd.dma_start(buv_sb[:], b_uv.rearrange("(h p) -> p h", p=128))

            # ---------------- P0: latentsT + qT (one pass over xT) --------
            # weights arrive via the gpsimd (SWDGE) queue so the sync queue
            # is free for x stripes; w_uk prefetches for P1.
            with tc.tile_pool(name="pkv", bufs=1) as pkvp:
              with tc.tile_pool(name="pq", bufs=1) as pqp:
                with tc.tile_pool(name="p0w", bufs=1) as wp, \
                     tc.tile_pool(name="p0x", bufs=2) as xp, \
                     tc.tile_pool(name="p0s", bufs=1) as sp, \
                     tc.tile_pool(name="p0ps", bufs=3, space="PSUM") as pps:
                    wdown_sb = wp.tile([128, n_dmc, DL], f32r, tag="wdown")
                    bdown_sb = wp.tile([128, n_lc], f32, tag="bdown")
                    nc.gpsimd.dma_start(wdown_sb[:], w_down.rearrange("(t p) l -> p t l", p=128))
                    nc.gpsimd.dma_start(bdown_sb[:], b_down.rearrange("(t p) -> p t", p=128))
                    wq_sb = pqp.tile([128, n_dmc, HH * DH], f32r, tag="wq")
                    bq_sb = pqp.tile([128, HH], f32, tag="bq")
                    nc.gpsimd.dma_start(wq_sb[:], w_q.rearrange("(t p) d -> p t d", p=128))
                    nc.gpsimd.dma_start(bq_sb[:], b_q.rearrange("(t p) -> p t", p=128))
                    wuk_sb = pkvp.tile([128, n_lc, HH * DH], f32r, tag="wuk")
                    buk_sb = pkvp.tile([128, HH], f32, tag="buk")
                    nc.gpsimd.dma_start(wuk_sb[:], w_uk.rearrange("(t p) d -> p t d", p=128))
                    nc.gpsimd.dma_start(buk_sb[:], b_uk.rearrange("(t p) -> p t", p=128))

                    for st in range(n_st):
                        ssl = bass.ts(st, ST)
                        xt = xp.tile([128, n_dmc, ST], f32r, tag="xt")
                        nc.sync.dma_start(xt[:], xT_r[:, :, ssl])
                        for lt in range(n_lc):
                            ps = pps.tile([128, ST], f32, tag="lat")
                            for c in range(n_dmc):
                                nc.tensor.matmul(ps[:], wdown_sb[:, c, bass.ts(lt, 128)],
                                                 xt[:, c, :], start=(c == 0), stop=(c == n_dmc - 1))
                            nc.scalar.activation(latentsT[:, lt, ssl], ps[:], AF.Identity,
                                                 bias=bdown_sb[:, lt:lt + 1])
                        qstage = sp.tile([128, HH, ST], f32r, tag="qs")
                        for ht in range(HH):
                            ps = pps.tile([128, ST], f32, tag="q")
                            for c in range(n_dmc):
                                nc.tensor.matmul(ps[:], wq_sb[:, c, bass.ts(ht, 128)],
                                                 xt[:, c, :], start=(c == 0), stop=(c == n_dmc - 1))
                            nc.scalar.activation(qstage[:, ht, :], ps[:], AF.Identity,
                                                 bias=bq_sb[:, ht:ht + 1])
                        nc.sync.dma_start(qTs_r[:, :, ssl], qstage[:])

              # ------------ P1: per 4-head group: K/V build + attention --
              with tc.tile_pool(name="p1kv", bufs=1) as kvp, \
                   tc.tile_pool(name="p1q", bufs=2) as qp, \
                   tc.tile_pool(name="p1e", bufs=2) as ep, \
                   tc.tile_pool(name="p1s", bufs=3) as sp2:
                    keysT = kvp.tile([128, 4, S], f32r, tag="keysT")
                    values = kvp.tile([128, n_kc, 512], f32r, tag="values")
                    wuv_sb = kvp.tile([128, n_lc, HH * DH], f32r, tag="wuv")
                    nc.gpsimd.dma_start(wuv_sb[:], w_uv.rearrange("(t p) d -> p t d", p=128))

                    for g in range(2):
                        gsl = bass.ds(g * 512, 512)
                        with tc.tile_pool(name=f"kvps{g}", bufs=2, space="PSUM") as kvps:
                            for dt_ in range(4):
                                for st in range(n_st):
                                    ps = kvps.tile([128, ST], f32, tag="kv")
                                    for lt in range(n_lc):
                                        nc.tensor.matmul(
                                            ps[:], wuk_sb[:, lt, bass.ds(g * 512 + dt_ * 128, 128)],
                                            latentsT[:, lt, bass.ts(st, ST)],
                                            start=(lt == 0), stop=(lt == n_lc - 1))
                                    nc.scalar.activation(keysT[:, dt_, bass.ts(st, ST)], ps[:],
                                                         AF.Identity,
                                                         bias=buk_sb[:, g * 4 + dt_:g * 4 + dt_ + 1])
                            for sc in range(n_kc):
                                ps = kvps.tile([128, 512], f32, tag="kv")
                                for lt in range(n_lc):
                                    nc.tensor.matmul(ps[:], latentsT[:, lt, bass.ts(sc, 128)],
                                                     wuv_sb[:, lt, gsl],
                                                     start=(lt == 0), stop=(lt == n_lc - 1))
                                nc.scalar.activation(values[:, sc, :], ps[:], AF.Identity)

                        with tc.tile_pool(name=f"scp{g}", bufs=3, space="PSUM") as scp, \
                             tc.tile_pool(name=f"avp{g}", bufs=1, space="PSUM") as avps, \
                             tc.tile_pool(name=f"bcp{g}", bufs=1, space="PSUM") as bcps:
                            for hl in range(4):
                                h = g * 4 + hl
                                qh = qp.tile([128, S], f32r, tag="qh")
                                nc.sync.dma_start(qh[:], qT_s[bass.ts(h, 128), :])
                                for qt in range(n_qt):
                                    qsl = bass.ts(qt, QT)
                                    eb = ep.tile([128, n_kc, QT], f32r, tag="eb")
                                    for blk in range(4):
                                        ps = scp.tile([128, 4, QT], f32, tag="sc")
                                        for c4 in range(4):
                                            c = blk * 4 + c4
                                            nc.tensor.matmul(ps[:, c4, :],
                                                             keysT[:, hl, bass.ts(c, 128)],
                                                             qh[:, qsl], start=True, stop=True)
                                        nc.scalar.activation(eb[:, blk * 4:(blk + 1) * 4, :],
                                                             ps[:], AF.Exp, scale=INV_SQRT_DH)
                                    avp = avps.tile([128, QT], f32, tag="av")
                                    for c in range(n_kc):
                                        nc.tensor.matmul(avp[:], values[:, c, bass.ts(hl, 128)],
                                                         eb[:, c, :], start=(c == 0),
                                                         stop=(c == n_kc - 1))
                                    bcp = bcps.tile([128, QT], f32, tag="bc")
                                    for c in range(n_kc):
                                        nc.tensor.matmul(bcp[0:1, :], onescol[:], eb[:, c, :],
                                                         start=(c == 0), stop=(c == n_kc - 1))
                                    recip = sp2.tile([1, QT], f32, tag="rc")
                                    nc.vector.reciprocal_approx_fast(recip[:], bcp[0:1, :])
                                    recipr = sp2.tile([1, QT], f32r, tag="rcr")
                                    nc.vector.tensor_scalar_add(recipr[:], recip[:], 0.0)
                                    nc.tensor.matmul(bcp[:], onesrow[:, :128], recipr[:],
                                                     start=True, stop=True)
                                    bcs = sp2.tile([128, QT], f32, tag="bcs")
                                    nc.vector.tensor_copy(bcs[:], bcp[:])
                                    tmp = sp2.tile([128, QT], f32, tag="tmp")
                                    nc.vector.tensor_mul(tmp[:], avp[:], bcs[:])
                                    ctxs = sp2.tile([128, QT], f32r, tag="ctxs")
                                    nc.vector.tensor_scalar_add(ctxs[:], tmp[:],
                                                                buv_sb[:, h:h + 1])
                                    nc.sync.dma_start(ctxT_s[bass.ts(h, 128), qsl], ctxs[:])

            # ---------------- P2: output projection (natural layout) ------
            # W_o resident in 4 chunks (prefetched in order); ctxT streamed
            # back per 128-query stripe.
            with tc.tile_pool(name="p2w", bufs=1) as wop, \
                 tc.tile_pool(name="p2c", bufs=3) as cp, \
                 tc.tile_pool(name="p2s", bufs=3) as osp, \
                 tc.tile_pool(name="p2ps", bufs=2, space="PSUM") as ops:
                wo_t = []
                for dmt in range(4):
                    w = wop.tile([128, HH, 512], f32r, tag=f"wo{dmt}")
                    nc.gpsimd.dma_start(w[:], wo_r[:, :, bass.ts(dmt, 512)])
                    wo_t.append(w)
                bo_sb = wop.tile([1, DM], f32r, tag="bo")
                nc.gpsimd.dma_start(bo_sb[:], b_o[None, :])
                for qt in range(S // 128):
                    qsl = bass.ts(qt, 128)
                    ctx_t = cp.tile([128, HH, 128], f32r, tag="ctxq")
                    nc.sync.dma_start(ctx_t[:], ctxs_r[:, :, qsl])
                    pst = [ops.tile([128, 512], f32, tag=f"o{dmt}", name=f"ps_o{dmt}") for dmt in range(4)]
                    for hc in range(HH):
                        for dmt in range(4):
                            nc.tensor.matmul(pst[dmt][:], ctx_t[:, hc, :],
                                             wo_t[dmt][:, hc, :], start=(hc == 0), stop=False)
                    for dmt in range(4):
                        dsl = bass.ts(dmt, 512)
                        nc.tensor.matmul(pst[dmt][:], onesrow[:, :128], bo_sb[:, dsl],
                                         start=False, stop=True)
                        ostage = osp.tile([128, 512], f32, tag="os")
                        nc.vector.tensor_copy(ostage[:], pst[dmt][:])
                        nc.sync.dma_start(out[qsl, dsl], ostage[:])

    nc.compile()
    return nc


def _get_nc():
    if "nc" not in _cache:
        _cache["nc"] = _build()
    return _cache["nc"]


def _in_maps(x, W_down, b_down, W_uk, b_uk, W_uv, b_uv, W_q, b_q, W_o, b_o):
    x = np.asarray(x, dtype=np.float32)
    zeros_bo = np.zeros_like(np.asarray(b_o))
    maps = []
    for c in range(N_CORES):
        b, hh = c // 2, c % 2
        hsl = slice(hh * HH * DH, (hh + 1) * HH * DH)
        maps.append({
            "xT": np.ascontiguousarray(x[b].T),
            "w_down": np.asarray(W_down),
            "w_q": np.ascontiguousarray(np.asarray(W_q)[:, hsl]),
            "w_uk": np.ascontiguousarray(np.asarray(W_uk)[:, hsl]),
            "w_uv": np.ascontiguousarray(np.asarray(W_uv)[:, hsl]),
            "w_o": np.ascontiguousarray(np.asarray(W_o)[hsl, :]),
            "b_down": np.asarray(b_down),
            "b_q": np.ascontiguousarray(np.asarray(b_q)[hsl]),
            "b_uk": np.ascontiguousarray(np.asarray(b_uk)[hsl]),
            "b_uv": np.ascontiguousarray(np.asarray(b_uv)[hsl]),
            "b_o": np.asarray(b_o) if hh == 0 else zeros_bo,
        })
    return maps


def kernel(x, W_down, b_down, W_uk, b_uk, W_uv, b_uv, W_q, b_q, W_o, b_o):
    from concourse.bass_utils import run_bass_kernel_spmd

    nc = _get_nc()
    maps = _in_maps(x, W_down, b_down, W_uk, b_uk, W_uv, b_uv, W_q, b_q, W_o, b_o)
    res = run_bass_kernel_spmd(nc, maps, list(range(N_CORES)))
    full = np.empty((B, S, DM), np.float32)
    for b in range(B):
        full[b] = res.results[2 * b]["out"] + res.results[2 * b + 1]["out"]
    return full
